# revision 37
# baseline (speedup 1.0000x reference)
"""Trainium2 Bass kernel for nn_MultiHeadAttention_44281112822190.

8 NeuronCores, pure data parallelism over the 8192 (b,s) rows: core c takes
rows [c*1024, (c+1)*1024) (batch b = c//2, s-offset (c%2)*1024). No
collectives; the host shards inputs and reassembles the output.

Math notes:
  - The reference applies RoPE to q and k, then contracts q.k at the SAME
    position (per-position head-head attention [B,S,H,H]). RoPE is an
    orthogonal per-position rotation applied identically to q and k, so it
    cancels exactly in the scores: (R q).(R k) = q.k. The kernel skips RoPE
    entirely (freqs inputs are unused).
  - The reference's "h-major flatten" transpose(0,2,1,3).reshape(B,S,-1) is a
    scramble: out[b, h*128 + s//16, (s%16)*128 + d] = att_out[b, s, h, d].
    Each scrambled row draws from 16 consecutive positions of one head, all
    inside one core's shard, so the output projection stays core-local.

Numerics: all matmul operands are fp16 with fp32 PSUM accumulation; the
final output is written fp16 (quantization ~2e-4, well inside the 2e-2
budget) and cast to fp32 on the host.

Per-core structure (one 1024-position block):
  1. Warmup: ~40 dependency-free matmuls on a zeroed tile run during the
     ~7us engine-boot window so the PE HAM clock-gate is released (2.4 GHz)
     by the time real data lands. Startup DMA triggers are issued in
     need-order on the sync queue (wq chunks + x interleaved) with
     constants and the second wq tile on the scalar (Activation) engine's
     parallel trigger queue.
  2. Q^T/K^T/V^T projections: stationary = host-transposed weight tiles,
     moving = host-transposed x; PSUM accumulated over 16 k-tiles, N=512.
     Block-interleaved output layout [128 d, 128 u, 16 h, 8 s] (position
     s = u*8+s_in): an 8-position attention slab is 128 contiguous cols
     (legal one-free-dim matmul operand) while copybacks write 8-element
     runs (~3x faster than stride-16). Copybacks alternate Scalar/Vector.
  3. Attention (64 pairs of two 8-position slabs): attn_a = score matmul
     [(h,w) x (g,w')] -> exp on ScalarE from PSUM -> fused mask-multiply +
     row-sum (DVE scalar_tensor_tensor accum_out) -> reciprocal ->
     normalize (GpSimd); attn_b = transpose att + V-slab on TensorE,
     attO^T slab = vT.T @ attT -> [d, (h,w)], scatter into attO halves.
     The V projection runs in two position-phases: phase A (positions
     0..511) interleaves all 32 attn_a of half 0; phase B interleaves
     their 32 attn_b (slabs lie in phase-A positions) -- no bunched
     attention drain, every chain hides under PE matmul streams. wv is
     streamed twice; projection-phase DMA bandwidth has 4x headroom.
     PE-queue ordering rule: every attn_b of half h must be emitted
     before the first final_t(h) (in-order queue -> a later-emitted
     producer would deadlock).
  4. Output projection in two position-halves (streams wo twice = 16MB,
     vs 4x for quarters, keeping total DMA below the ~358GB/s per-core
     cap alongside fp16 output writes); N=512 matmuls; pass 0 interleaves
     attention half 1 two pairs per chunk; wot tiles prefetched two
     chunks ahead through the 3-buffer weight pool.
Host reassembles the scrambled rows into the final [4, 2048, 2048] output.
"""

import os
import sys

sys.path.insert(0, "/opt/trn_rl_repo")

import numpy as np

import concourse.bacc as bacc
import concourse.mybir as mybir
import concourse.tile as tile
from concourse.bass_utils import run_bass_kernel_spmd

F32 = mybir.dt.float32
F16 = mybir.dt.float16
AF = mybir.ActivationFunctionType
ALU = mybir.AluOpType

B, S, E, H, D = 4, 2048, 2048, 16, 128
NCORES = 8
SCALE = 1.0 / float(np.sqrt(D))

_CACHE = {}
LAST_EXEC_NS = None


def _build():
    nc = bacc.Bacc(trn_type="TRN2", target_bir_lowering=False)

    xt = nc.dram_tensor("xt", [16, 128, 1024], F16, kind="ExternalInput")
    wqt = nc.dram_tensor("wqt", [E, E], F16, kind="ExternalInput")
    wkt = nc.dram_tensor("wkt", [E, E], F16, kind="ExternalInput")
    wvt = nc.dram_tensor("wvt", [E, E], F16, kind="ExternalInput")
    wot = nc.dram_tensor("wot", [E, E], F16, kind="ExternalInput")
    bqt = nc.dram_tensor("bqt", [128, 16], F32, kind="ExternalInput")
    bkt = nc.dram_tensor("bkt", [128, 16], F32, kind="ExternalInput")
    bvt = nc.dram_tensor("bvt", [128, 16], F32, kind="ExternalInput")
    bot = nc.dram_tensor("bot", [128, 16], F32, kind="ExternalInput")
    mask01 = nc.dram_tensor("mask01", [128, 128], F32, kind="ExternalInput")
    ident = nc.dram_tensor("ident", [128, 128], F16, kind="ExternalInput")
    out = nc.dram_tensor("out", [16, 128, 1024], F16, kind="ExternalOutput")

    with tile.TileContext(nc) as tc:
        with (
            tc.tile_pool(name="const", bufs=1) as cp,
            tc.tile_pool(name="xp", bufs=1) as xp,
            tc.tile_pool(name="qkv", bufs=1) as qkvp,
            tc.tile_pool(name="aop", bufs=1) as aop,
            tc.tile_pool(name="wp", bufs=3) as wp,
            tc.tile_pool(name="gp", bufs=5) as gp,
            tc.tile_pool(name="op", bufs=2) as op,
            tc.tile_pool(name="pp", bufs=3, space="PSUM") as pp,
            tc.tile_pool(name="pa", bufs=3, space="PSUM") as pa,
            tc.tile_pool(name="pb", bufs=2, space="PSUM") as pb,
        ):
            # --- PE warmup: release the HAM clock gate during boot.
            # Dependency-free matmuls emitted into the first real projection
            # PSUM tile (the real k=0 matmul's start=True overwrites them, and
            # the tile has readers so BIR verification passes). ---
            wz = cp.tile([128, 128], F16, tag="wz")
            nc.gpsimd.memset(wz[:], 0.0)
            NWARM = 40

            # --- startup DMAs, ordered by need-time across two trigger
            # queues (sync + scalar run their DMA triggers in parallel) ---
            xtb_c = []
            for kc in range(4):
                xc = xp.tile([128, 4, 1024], F16, tag=f"xtb{kc}", name=f"xtb{kc}")
                xtb_c.append(xc)

            wtile0 = wp.tile([128, 16, 256], F16, tag="w", name="wt0")
            wtile1 = wp.tile([128, 16, 256], F16, tag="w", name="wt1")
            # sync queue carries the startup-critical stream in need-order;
            # trigger issue itself serializes (~0.7-1.2us per DMA), which
            # staggers the transfers so the first chunks aren't bandwidth-
            # starved by later ones
            nc.sync.dma_start(
                wtile0[:, 0:4, :],
                wqt[0:512, 0:256].rearrange("(k p) c -> p k c", p=128),
            )
            nc.sync.dma_start(
                xtb_c[0][:, :, 0:512],
                xt[0:4, :, 0:512].rearrange("k p s -> p k s"),
            )
            nc.sync.dma_start(
                wtile0[:, 4:16, :],
                wqt[512:2048, 0:256].rearrange("(k p) c -> p k c", p=128),
            )
            for kc in range(1, 4):
                nc.sync.dma_start(
                    xtb_c[kc][:, :, 0:512],
                    xt[4 * kc : 4 * kc + 4, :, 0:512].rearrange("k p s -> p k s"),
                )
            # scalar queue (parallel): x h1 chunks, then constants, then the
            # second wq tile so the sync queue stays clear for the x stream
            for kc in range(4):
                nc.scalar.dma_start(
                    xtb_c[kc][:, :, 512:1024],
                    xt[4 * kc : 4 * kc + 4, :, 512:1024].rearrange("k p s -> p k s"),
                )
            bias_sb = {}
            for name, t_ in (("bq", bqt), ("bk", bkt), ("bv", bvt), ("bo", bot)):
                b_sb = cp.tile([128, 16], F32, tag=name)
                nc.scalar.dma_start(b_sb[:], t_[:, :])
                bias_sb[name] = b_sb
            mask_sb = cp.tile([128, 128], F32, tag="mask")
            id_sb = cp.tile([128, 128], F16, tag="id")
            nc.scalar.dma_start(mask_sb[:], mask01[:, :])
            nc.scalar.dma_start(id_sb[:], ident[:, :])
            nc.scalar.dma_start(
                wtile1[:],
                wqt[:, 256:512].rearrange("(k p) c -> p k c", p=128),
            )

            # --- Q/K/V projections -> [128 d, 128 u, 16 h, 8 s] fp16 ---
            # Block-interleaved layout: position s = u*8 + s_in. An 8-position
            # attention slab is tile[:, u] = 128 CONTIGUOUS cols (h-major), a
            # legal one-free-dim matmul operand, while the PSUM->SBUF
            # copyback writes 8-contiguous runs instead of stride-16 single
            # elements (which measured ~3.6x slower on DVE).
            qb = qkvp.tile([128, 128, 16, 8], F16, tag="qb")
            kb = qkvp.tile([128, 128, 16, 8], F16, tag="kb")
            vb = qkvp.tile([128, 128, 16, 8], F16, tag="vb")

            def copyback(dst, posh, t, ps, bias):
                # positions posh*512..posh*512+512 of feature chunk (head) t;
                # alternate Scalar/Vector by t so neither engine saturates
                d_ap = dst[:, 64 * posh : 64 * posh + 64, t, :]
                s_ap = ps[:].rearrange("p (u s) -> p u s", s=8)
                if t % 2 == 0:
                    nc.scalar.activation(
                        d_ap, s_ap, AF.Identity, bias=bias_sb[bias][:, t : t + 1]
                    )
                else:
                    nc.vector.tensor_scalar_add(
                        d_ap, s_ap, bias_sb[bias][:, t : t + 1]
                    )

            for wdram, bias, dst in (
                (wqt, "bq", qb),
                (wkt, "bk", kb),
            ):
                for t2 in range(8):
                    if wdram is wqt and t2 == 0:
                        wtile = wtile0
                    elif wdram is wqt and t2 == 1:
                        wtile = wtile1
                    else:
                        wtile = wp.tile([128, 16, 256], F16, tag="w")
                        nc.sync.dma_start(
                            wtile[:],
                            wdram[:, t2 * 256 : (t2 + 1) * 256].rearrange(
                                "(k p) c -> p k c", p=128
                            ),
                        )
                    for half in range(2):
                        t = 2 * t2 + half
                        psA = pp.tile([128, 512], F32, tag="pp")
                        psB = pp.tile([128, 512], F32, tag="pp")
                        if wdram is wqt and t == 0:
                            for i in range(NWARM):
                                nc.tensor.matmul(
                                    psA[:, 0:128], wz[:], wz[:],
                                    start=(i == 0), stop=(i == NWARM - 1),
                                )
                        for k in range(16):
                            w_ap = wtile[:, k, half * 128 : half * 128 + 128]
                            nc.tensor.matmul(
                                psA[:], w_ap, xtb_c[k // 4][:, k % 4, 0:512],
                                start=(k == 0), stop=(k == 15),
                            )
                        for k in range(16):
                            w_ap = wtile[:, k, half * 128 : half * 128 + 128]
                            nc.tensor.matmul(
                                psB[:], w_ap, xtb_c[k // 4][:, k % 4, 512:1024],
                                start=(k == 0), stop=(k == 15),
                            )
                        copyback(dst, 0, t, psA, bias)
                        copyback(dst, 1, t, psB, bias)

            # --- attention (pairs) + overlapped output projection halves ---
            # attO half tiles: [128 d, 16 sl, 512] with col = u_local*16 + h,
            # u_local = (s//16) % 32. Two position-halves instead of four
            # quarters so the out-projection streams wot only twice (16MB
            # instead of 32MB) and stays well under the per-core DMA
            # bandwidth cap alongside the fp16 output writes.
            attO_h = [
                aop.tile([128, 16, 512], F16, tag=f"attO{q}", name=f"attO{q}")
                for q in range(2)
            ]

            def slab(t_, u):
                # 8-position slab u: [128, 16 h, 8 s] contiguous 128 cols,
                # h-major enumeration -> score partitions/cols are (h*8 + w)
                return t_[:, u, :, :]

            # attention is split in two stages with a 2-pair lag so the
            # softmax chain (exp->mask+sum->recip->normalize) of pair p runs
            # while the PE does other pairs' matmuls instead of stalling on it
            att2_t = {}

            def attn_a(P2):
                G = 2 * P2
                ga = pa.tile([128, 512], F32, tag="ga")
                for j in range(2):
                    nc.tensor.matmul(
                        ga[:, 128 * j : 128 * j + 128],
                        slab(qb, G + j),
                        slab(kb, G + j),
                        start=True, stop=True,
                    )
                # f16 exp values: att weights are f16 downstream anyway, and
                # the f32->f16 squeeze halves this pool's footprint
                e2 = gp.tile([128, 256], F16, tag="e2", bufs=4)
                nc.scalar.activation(e2[:], ga[:, 0:256], AF.Exp, scale=SCALE)
                em2 = e2[:].rearrange("p (g c) -> p g c", g=2)
                den2 = gp.tile([128, 2], F32, tag="den2")
                for j in range(2):
                    nc.vector.scalar_tensor_tensor(
                        em2[:, j, :], e2[:, 128 * j : 128 * j + 128], 1.0,
                        mask_sb[:], ALU.bypass, ALU.mult,
                        accum_out=den2[:, j : j + 1],
                    )
                rec2 = gp.tile([128, 2], F32, tag="rec2")
                nc.vector.reciprocal(rec2[:], den2[:])
                # all 32 half-0 scores run inside the V projection; up to 33
                # att2 tiles are outstanding before their attn_b drains
                att2 = gp.tile([128, 2, 128], F16, tag="att2", bufs=32)
                nc.gpsimd.tensor_tensor(
                    att2[:], em2, rec2[:].unsqueeze(2).to_broadcast([128, 2, 128]),
                    ALU.mult,
                )
                att2_t[P2] = att2

            def attn_b(P2):
                G = 2 * P2
                att2 = att2_t.pop(P2)
                tr = pb.tile([128, 512], F16, tag="tr")
                for j in range(2):
                    nc.tensor.transpose(
                        tr[:, 128 * j : 128 * j + 128], att2[:, j, :], id_sb[:]
                    )
                    nc.tensor.transpose(
                        tr[:, 256 + 128 * j : 384 + 128 * j], slab(vb, G + j),
                        id_sb[:],
                    )
                # PSUM->SBUF copy alternates Scalar/Vector by pair parity so
                # a bunched attn_b drain isn't bound on one engine
                trsb = gp.tile([128, 512], F16, tag="trsb", bufs=2)
                if P2 % 2 == 0:
                    nc.scalar.activation(trsb[:], tr[:], AF.Copy)
                else:
                    nc.vector.tensor_copy(trsb[:], tr[:])
                po = pa.tile([128, 512], F32, tag="ga", name="po")
                for j in range(2):
                    nc.tensor.matmul(
                        po[:, 128 * j : 128 * j + 128],
                        trsb[:, 256 + 128 * j : 384 + 128 * j],
                        trsb[:, 128 * j : 128 * j + 128],
                        start=True, stop=True,
                    )
                # scatter: psum cols (a, h, w) -> attO_h[u_hi][:, a*8+w, u_lo*16+h]
                # (po cols are h-major inside each slab a, matching the
                # h-major slab enumeration of the block-interleaved layout)
                u_hi, u_lo = P2 // 32, P2 % 32
                dst = attO_h[u_hi][:].rearrange(
                    "p (a w) (u h) -> p a h w u", a=2, h=16
                )[:, :, :, :, u_lo]
                if P2 % 2 == 0:
                    nc.vector.tensor_copy(dst, po[:, 0:256])
                else:
                    nc.scalar.activation(dst, po[:, 0:256], AF.Copy)

            pending_b = []

            def attn_pair(P2):
                # scores run 2 iterations ahead of the quarter window so the
                # last pairs' softmax chains are done before the flush drains
                # them; P2 >= 64 iterations only drain.
                if P2 < 64:
                    attn_a(P2)
                    pending_b.append(P2)
                    if len(pending_b) > 2:
                        attn_b(pending_b.pop(0))
                elif pending_b:
                    attn_b(pending_b.pop(0))

            def flush_b(upto):
                # attO quarter q must be complete before any final_t(q) is
                # emitted: the PE queue is in-order, so a later-emitted
                # attn_b could never satisfy an earlier final_t (deadlock)
                while pending_b and pending_b[0] < upto:
                    attn_b(pending_b.pop(0))

            def final_w_dma(t2):
                wtile = wp.tile([128, 16, 256], F16, tag="w")
                nc.sync.dma_start(
                    wtile[:],
                    wot[:, t2 * 256 : (t2 + 1) * 256].rearrange(
                        "(k p) c -> p k c", p=128
                    ),
                )
                return wtile

            def final_t(hf, t2, half, wtile):
                t = 2 * t2 + half
                ps = pp.tile([128, 512], F32, tag="pp")
                for sl in range(16):
                    nc.tensor.matmul(
                        ps[:],
                        wtile[:, sl, half * 128 : half * 128 + 128],
                        attO_h[hf][:, sl, :],
                        start=(sl == 0), stop=(sl == 15),
                    )
                ob = op.tile([128, 512], F16, tag="ob")
                nc.vector.tensor_scalar_add(
                    ob[:], ps[:], bias_sb["bo"][:, t : t + 1]
                )
                nc.sync.dma_start(
                    out[t, :, hf * 512 : hf * 512 + 512], ob[:]
                )

            # V projection in two position-phases so attention half 0
            # completes entirely under it with no bunched drain:
            #   phase A (positions 0..511): one psA group per (t2,half) with
            #     both of that iteration's attn_a score pairs interleaved
            #     (pairs 0..31 need only q/k);
            #   phase B (positions 512..1023): psB groups with two attn_b
            #     stages per iteration (their slabs lie in phase A's
            #     positions, complete by then).
            # wv is streamed twice (16MB total) -- the projection-phase DMA
            # bandwidth has 4x headroom, PE cycles are the binding resource.
            wt_pre = []
            for phase in range(2):
                for t2 in range(8):
                    if phase == 1 and t2 == 7:
                        # prefetch pass 0's first two wot tiles here: late
                        # enough that the 3-buffer rotation doesn't stall
                        # phase B's wv stream, early enough (~14us of cover)
                        # that pass 0 never waits on them
                        wt_pre.append(final_w_dma(0))
                        wt_pre.append(final_w_dma(1))
                    wtile = wp.tile([128, 16, 256], F16, tag="w")
                    nc.sync.dma_start(
                        wtile[:],
                        wvt[:, t2 * 256 : (t2 + 1) * 256].rearrange(
                            "(k p) c -> p k c", p=128
                        ),
                    )
                    for half in range(2):
                        t = 2 * t2 + half
                        ps = pp.tile([128, 512], F32, tag="pp")
                        for k in range(16):
                            w_ap = wtile[:, k, half * 128 : half * 128 + 128]
                            nc.tensor.matmul(
                                ps[:], w_ap,
                                xtb_c[k // 4][:, k % 4, 512 * phase : 512 * phase + 512],
                                start=(k == 0), stop=(k == 15),
                            )
                        copyback(vb, phase, t, ps, "bv")
                        if phase == 0:
                            attn_a(2 * t)
                            pending_b.append(2 * t)
                            attn_a(2 * t + 1)
                            pending_b.append(2 * t + 1)
                        else:
                            attn_b(pending_b.pop(0))
                            attn_b(pending_b.pop(0))
            # attO half 0 is complete (flush is a no-op safety net)
            wt_next, wt_next2 = wt_pre
            flush_b(32)
            # pairs 32..35 scored early so the +4-shifted interleave window
            # covers every pair exactly once and the last attn_b scatters
            # land two chunks before pass 1 reads attO half 1
            for p_early in (32, 33, 34, 35):
                attn_a(p_early)
                pending_b.append(p_early)
            # pass 0 over attO half 0, interleaved 2:1 with the 32 attention
            # pairs of half 1; wot tiles prefetched two chunks ahead
            for t2 in range(8):
                wtile = wt_next
                wt_next = wt_next2
                wt_next2 = final_w_dma((t2 + 2) % 8)
                for half in range(2):
                    final_t(0, t2, half, wtile)
                    attn_pair(32 + 2 * (2 * t2 + half) + 4)
                    attn_pair(32 + 2 * (2 * t2 + half) + 5)
            flush_b(64)
            # pass 1 over attO half 1 has no attention to hide behind: two
            # wot tiles stay in flight so its matmuls never wait on the stream
            for t2 in range(8):
                wtile = wt_next
                wt_next = wt_next2
                if t2 < 6:
                    wt_next2 = final_w_dma(t2 + 2)
                for half in range(2):
                    final_t(1, t2, half, wtile)

    nc.compile()
    return nc


def _get_nc():
    if "nc" not in _CACHE:
        _CACHE["nc"] = _build()
    return _CACHE["nc"]


def make_in_maps(inputs):
    x = np.ascontiguousarray(np.asarray(inputs["x"], dtype=np.float32))
    ws = {k: np.asarray(inputs[k], dtype=np.float32) for k in ("wq", "wk", "wv", "wo")}
    bs = {k: np.asarray(inputs[k], dtype=np.float32) for k in ("bq", "bk", "bv", "bo")}

    xf = x.reshape(B * S, E)
    f16 = lambda a: np.ascontiguousarray(a).astype(np.float16)
    btile = lambda b: np.ascontiguousarray(b.reshape(16, 128).T)
    # score partitions/cols are (h*8 + w) h-major under the block-interleaved
    # qkv layout; same-position mask is w_q == w_k i.e. p%8 == c%8
    ii = np.arange(128) % 8
    mask01 = (ii[:, None] == ii[None, :]).astype(np.float32)
    common = {
        "wqt": f16(ws["wq"].T), "wkt": f16(ws["wk"].T),
        "wvt": f16(ws["wv"].T), "wot": f16(ws["wo"].T),
        "bqt": btile(bs["bq"]), "bkt": btile(bs["bk"]),
        "bvt": btile(bs["bv"]), "bot": btile(bs["bo"]),
        "mask01": mask01, "ident": np.eye(128, dtype=np.float16),
    }
    in_maps = []
    for c in range(NCORES):
        xt_c = f16(xf[c * 1024 : (c + 1) * 1024].T).reshape(16, 128, 1024)
        in_maps.append({"xt": xt_c, **common})
    return in_maps


def assemble(results):
    out = np.empty((B, S, E), np.float32)
    for c in range(NCORES):
        O = results[c]["out"].astype(np.float32)  # [16 t, 128 p, 1024]; col = u*16 + h
        Oc = O.reshape(E, 64, 16)  # [j, u, h]
        tgt = out[c // 2].reshape(16, 128, E)
        v0 = (c % 2) * 64
        tgt[:, v0 : v0 + 64, :] = Oc.transpose(2, 1, 0)
    return out


def kernel(**inputs):
    global LAST_EXEC_NS
    nc = _get_nc()
    res = run_bass_kernel_spmd(nc, make_in_maps(inputs), core_ids=list(range(NCORES)))
    LAST_EXEC_NS = res.exec_time_ns
    return assemble(res.results)


# revision 43
# speedup vs baseline: 1.0085x; 1.0085x over previous
"""Trainium2 Bass kernel for nn_MultiHeadAttention_44281112822190.

8 NeuronCores, pure data parallelism over the 8192 (b,s) rows: core c takes
rows [c*1024, (c+1)*1024) (batch b = c//2, s-offset (c%2)*1024). No
collectives; the host shards inputs and reassembles the output.

Math notes:
  - The reference applies RoPE to q and k, then contracts q.k at the SAME
    position (per-position head-head attention [B,S,H,H]). RoPE is an
    orthogonal per-position rotation applied identically to q and k, so it
    cancels exactly in the scores: (R q).(R k) = q.k. The kernel skips RoPE
    entirely (freqs inputs are unused).
  - The reference's "h-major flatten" transpose(0,2,1,3).reshape(B,S,-1) is a
    scramble: out[b, h*128 + s//16, (s%16)*128 + d] = att_out[b, s, h, d].
    Each scrambled row draws from 16 consecutive positions of one head, all
    inside one core's shard, so the output projection stays core-local.

Numerics: all matmul operands are fp16 with fp32 PSUM accumulation; the
final output is written fp16 (quantization ~2e-4, well inside the 2e-2
budget) and cast to fp32 on the host.

Per-core structure (one 1024-position block):
  1. Warmup: ~40 dependency-free matmuls on a zeroed tile run during the
     ~7us engine-boot window so the PE HAM clock-gate is released (2.4 GHz)
     by the time real data lands. Startup DMA triggers are split across
     two parallel trigger queues: sync carries the first wq tile (in two
     chunks) + the x position-half-0 stream, the scalar (Activation)
     engine carries x half-1 + biases/constants.
  2. Q^T/K^T/V^T projections: stationary = host-transposed weight tiles,
     moving = host-transposed x; PSUM accumulated over 16 k-tiles, N=512.
     Block-interleaved output layout [128 d, 128 u, 16 h, 8 s] (position
     s = u*8+s_in): an 8-position attention slab is 128 contiguous cols
     (legal one-free-dim matmul operand) while copybacks write 8-element
     runs (~3x faster than stride-16). Copybacks alternate Scalar/Vector.
  3. Attention (64 pairs of two 8-position slabs): attn_a = score matmul
     [(h,w) x (g,w')] -> exp on ScalarE from PSUM -> fused mask-multiply +
     row-sum (DVE scalar_tensor_tensor accum_out) -> reciprocal ->
     normalize (GpSimd); attn_b = transpose att + V-slab on TensorE,
     attO^T slab = vT.T @ attT -> [d, (h,w)], scatter into attO halves.
     The V projection runs in two position-phases: phase A (positions
     0..511) interleaves all 32 attn_a of half 0; phase B interleaves
     their 32 attn_b (slabs lie in phase-A positions) -- no bunched
     attention drain, every chain hides under PE matmul streams. wv is
     streamed twice; projection-phase DMA bandwidth has 4x headroom.
     PE-queue ordering rule: every attn_b of half h must be emitted
     before the first final_t(h) (in-order queue -> a later-emitted
     producer would deadlock).
  4. Output projection in two position-halves (streams wo twice = 16MB,
     vs 4x for quarters, keeping total DMA below the ~358GB/s per-core
     cap alongside fp16 output writes); N=512 matmuls; pass 0 interleaves
     attention half 1 two pairs per chunk; wot tiles prefetched two
     chunks ahead through the 3-buffer weight pool.
Host reassembles the scrambled rows into the final [4, 2048, 2048] output.
"""

import os
import sys

sys.path.insert(0, "/opt/trn_rl_repo")

import numpy as np

import concourse.bacc as bacc
import concourse.mybir as mybir
import concourse.tile as tile
from concourse.bass_utils import run_bass_kernel_spmd

F32 = mybir.dt.float32
F16 = mybir.dt.float16
AF = mybir.ActivationFunctionType
ALU = mybir.AluOpType

B, S, E, H, D = 4, 2048, 2048, 16, 128
NCORES = 8
SCALE = 1.0 / float(np.sqrt(D))

_CACHE = {}
LAST_EXEC_NS = None


def _build():
    nc = bacc.Bacc(trn_type="TRN2", target_bir_lowering=False)

    xt = nc.dram_tensor("xt", [16, 128, 1024], F16, kind="ExternalInput")
    wqt = nc.dram_tensor("wqt", [E, E], F16, kind="ExternalInput")
    wkt = nc.dram_tensor("wkt", [E, E], F16, kind="ExternalInput")
    wvt = nc.dram_tensor("wvt", [E, E], F16, kind="ExternalInput")
    wot = nc.dram_tensor("wot", [E, E], F16, kind="ExternalInput")
    bqt = nc.dram_tensor("bqt", [128, 16], F32, kind="ExternalInput")
    bkt = nc.dram_tensor("bkt", [128, 16], F32, kind="ExternalInput")
    bvt = nc.dram_tensor("bvt", [128, 16], F32, kind="ExternalInput")
    bot = nc.dram_tensor("bot", [128, 16], F32, kind="ExternalInput")
    mask01 = nc.dram_tensor("mask01", [128, 128], F32, kind="ExternalInput")
    ident = nc.dram_tensor("ident", [128, 128], F16, kind="ExternalInput")
    out = nc.dram_tensor("out", [16, 128, 1024], F16, kind="ExternalOutput")

    with tile.TileContext(nc) as tc:
        with (
            tc.tile_pool(name="const", bufs=1) as cp,
            tc.tile_pool(name="xp", bufs=1) as xp,
            tc.tile_pool(name="qkv", bufs=1) as qkvp,
            tc.tile_pool(name="aop", bufs=1) as aop,
            tc.tile_pool(name="wp", bufs=3) as wp,
            tc.tile_pool(name="gp", bufs=5) as gp,
            tc.tile_pool(name="op", bufs=2) as op,
            tc.tile_pool(name="pp", bufs=3, space="PSUM") as pp,
            tc.tile_pool(name="pa", bufs=3, space="PSUM") as pa,
            tc.tile_pool(name="pb", bufs=2, space="PSUM") as pb,
        ):
            # --- PE warmup: release the HAM clock gate during boot.
            # Dependency-free matmuls emitted into the first real projection
            # PSUM tile (the real k=0 matmul's start=True overwrites them, and
            # the tile has readers so BIR verification passes). ---
            wz = cp.tile([128, 128], F16, tag="wz")
            nc.gpsimd.memset(wz[:], 0.0)
            NWARM = 40

            # --- startup DMAs, ordered by need-time across two trigger
            # queues (sync + scalar run their DMA triggers in parallel) ---
            xtb_c = []
            for kc in range(4):
                xc = xp.tile([128, 4, 1024], F16, tag=f"xtb{kc}", name=f"xtb{kc}")
                xtb_c.append(xc)

            wtile0 = wp.tile([128, 16, 256], F16, tag="w", name="wt0")
            # sync queue: first wq chunk (k 0-3), x h0 kc0, rest of wq0,
            # remaining x h0 chunks
            nc.sync.dma_start(
                wtile0[:, 0:4, :],
                wqt[0:512, 0:256].rearrange("(k p) c -> p k c", p=128),
            )
            nc.sync.dma_start(
                xtb_c[0][:, :, 0:512],
                xt[0:4, :, 0:512].rearrange("k p s -> p k s"),
            )
            nc.sync.dma_start(
                wtile0[:, 4:16, :],
                wqt[512:2048, 0:256].rearrange("(k p) c -> p k c", p=128),
            )
            for kc in range(1, 4):
                nc.sync.dma_start(
                    xtb_c[kc][:, :, 0:512],
                    xt[4 * kc : 4 * kc + 4, :, 0:512].rearrange("k p s -> p k s"),
                )
            # scalar queue (parallel): x h1 chunks, then biases + constants
            for kc in range(4):
                nc.scalar.dma_start(
                    xtb_c[kc][:, :, 512:1024],
                    xt[4 * kc : 4 * kc + 4, :, 512:1024].rearrange("k p s -> p k s"),
                )
            bias_sb = {}
            for name, t_ in (("bq", bqt), ("bk", bkt), ("bv", bvt), ("bo", bot)):
                b_sb = cp.tile([128, 16], F32, tag=name)
                nc.scalar.dma_start(b_sb[:], t_[:, :])
                bias_sb[name] = b_sb
            mask_sb = cp.tile([128, 128], F32, tag="mask")
            id_sb = cp.tile([128, 128], F16, tag="id")
            nc.scalar.dma_start(mask_sb[:], mask01[:, :])
            nc.scalar.dma_start(id_sb[:], ident[:, :])

            # --- Q/K/V projections -> [128 d, 128 u, 16 h, 8 s] fp16 ---
            # Block-interleaved layout: position s = u*8 + s_in. An 8-position
            # attention slab is tile[:, u] = 128 CONTIGUOUS cols (h-major), a
            # legal one-free-dim matmul operand, while the PSUM->SBUF
            # copyback writes 8-contiguous runs instead of stride-16 single
            # elements (which measured ~3.6x slower on DVE).
            qb = qkvp.tile([128, 128, 16, 8], F16, tag="qb")
            kb = qkvp.tile([128, 128, 16, 8], F16, tag="kb")
            vb = qkvp.tile([128, 128, 16, 8], F16, tag="vb")

            def copyback(dst, posh, t, ps, bias):
                # positions posh*512..posh*512+512 of feature chunk (head) t;
                # alternate Scalar/Vector by t so neither engine saturates
                d_ap = dst[:, 64 * posh : 64 * posh + 64, t, :]
                s_ap = ps[:].rearrange("p (u s) -> p u s", s=8)
                if t % 2 == 0:
                    nc.scalar.activation(
                        d_ap, s_ap, AF.Identity, bias=bias_sb[bias][:, t : t + 1]
                    )
                else:
                    nc.vector.tensor_scalar_add(
                        d_ap, s_ap, bias_sb[bias][:, t : t + 1]
                    )

            for wdram, bias, dst in (
                (wqt, "bq", qb),
                (wkt, "bk", kb),
            ):
                for t2 in range(8):
                    if wdram is wqt and t2 == 0:
                        wtile = wtile0
                    else:
                        wtile = wp.tile([128, 16, 256], F16, tag="w")
                        nc.sync.dma_start(
                            wtile[:],
                            wdram[:, t2 * 256 : (t2 + 1) * 256].rearrange(
                                "(k p) c -> p k c", p=128
                            ),
                        )
                    for half in range(2):
                        t = 2 * t2 + half
                        psA = pp.tile([128, 512], F32, tag="pp")
                        psB = pp.tile([128, 512], F32, tag="pp")
                        if wdram is wqt and t == 0:
                            for i in range(NWARM):
                                nc.tensor.matmul(
                                    psA[:, 0:128], wz[:], wz[:],
                                    start=(i == 0), stop=(i == NWARM - 1),
                                )
                        for k in range(16):
                            w_ap = wtile[:, k, half * 128 : half * 128 + 128]
                            nc.tensor.matmul(
                                psA[:], w_ap, xtb_c[k // 4][:, k % 4, 0:512],
                                start=(k == 0), stop=(k == 15),
                            )
                        for k in range(16):
                            w_ap = wtile[:, k, half * 128 : half * 128 + 128]
                            nc.tensor.matmul(
                                psB[:], w_ap, xtb_c[k // 4][:, k % 4, 512:1024],
                                start=(k == 0), stop=(k == 15),
                            )
                        copyback(dst, 0, t, psA, bias)
                        copyback(dst, 1, t, psB, bias)

            # --- attention (pairs) + overlapped output projection halves ---
            # attO half tiles: [128 d, 16 sl, 512] with col = u_local*16 + h,
            # u_local = (s//16) % 32. Two position-halves instead of four
            # quarters so the out-projection streams wot only twice (16MB
            # instead of 32MB) and stays well under the per-core DMA
            # bandwidth cap alongside the fp16 output writes.
            attO_h = [
                aop.tile([128, 16, 512], F16, tag=f"attO{q}", name=f"attO{q}")
                for q in range(2)
            ]

            def slab(t_, u):
                # 8-position slab u: [128, 16 h, 8 s] contiguous 128 cols,
                # h-major enumeration -> score partitions/cols are (h*8 + w)
                return t_[:, u, :, :]

            # attention is split in two stages with a 2-pair lag so the
            # softmax chain (exp->mask+sum->recip->normalize) of pair p runs
            # while the PE does other pairs' matmuls instead of stalling on it
            att2_t = {}

            def attn_a(P2):
                G = 2 * P2
                ga = pa.tile([128, 512], F32, tag="ga")
                for j in range(2):
                    nc.tensor.matmul(
                        ga[:, 128 * j : 128 * j + 128],
                        slab(qb, G + j),
                        slab(kb, G + j),
                        start=True, stop=True,
                    )
                # f16 exp values: att weights are f16 downstream anyway, and
                # the f32->f16 squeeze halves this pool's footprint
                e2 = gp.tile([128, 256], F16, tag="e2", bufs=4)
                nc.scalar.activation(e2[:], ga[:, 0:256], AF.Exp, scale=SCALE)
                em2 = e2[:].rearrange("p (g c) -> p g c", g=2)
                den2 = gp.tile([128, 2], F32, tag="den2")
                for j in range(2):
                    nc.vector.scalar_tensor_tensor(
                        em2[:, j, :], e2[:, 128 * j : 128 * j + 128], 1.0,
                        mask_sb[:], ALU.bypass, ALU.mult,
                        accum_out=den2[:, j : j + 1],
                    )
                rec2 = gp.tile([128, 2], F32, tag="rec2")
                nc.vector.reciprocal(rec2[:], den2[:])
                # all 32 half-0 scores run inside the V projection; up to 33
                # att2 tiles are outstanding before their attn_b drains
                att2 = gp.tile([128, 2, 128], F16, tag="att2", bufs=32)
                nc.gpsimd.tensor_tensor(
                    att2[:], em2, rec2[:].unsqueeze(2).to_broadcast([128, 2, 128]),
                    ALU.mult,
                )
                att2_t[P2] = att2

            def attn_b(P2):
                G = 2 * P2
                att2 = att2_t.pop(P2)
                tr = pb.tile([128, 512], F16, tag="tr")
                for j in range(2):
                    nc.tensor.transpose(
                        tr[:, 128 * j : 128 * j + 128], att2[:, j, :], id_sb[:]
                    )
                    nc.tensor.transpose(
                        tr[:, 256 + 128 * j : 384 + 128 * j], slab(vb, G + j),
                        id_sb[:],
                    )
                # PSUM->SBUF copy alternates Scalar/Vector by pair parity so
                # a bunched attn_b drain isn't bound on one engine
                trsb = gp.tile([128, 512], F16, tag="trsb", bufs=2)
                if P2 % 2 == 0:
                    nc.scalar.activation(trsb[:], tr[:], AF.Copy)
                else:
                    nc.vector.tensor_copy(trsb[:], tr[:])
                po = pa.tile([128, 512], F32, tag="ga", name="po")
                for j in range(2):
                    nc.tensor.matmul(
                        po[:, 128 * j : 128 * j + 128],
                        trsb[:, 256 + 128 * j : 384 + 128 * j],
                        trsb[:, 128 * j : 128 * j + 128],
                        start=True, stop=True,
                    )
                # scatter: psum cols (a, h, w) -> attO_h[u_hi][:, a*8+w, u_lo*16+h]
                # (po cols are h-major inside each slab a, matching the
                # h-major slab enumeration of the block-interleaved layout)
                u_hi, u_lo = P2 // 32, P2 % 32
                dst = attO_h[u_hi][:].rearrange(
                    "p (a w) (u h) -> p a h w u", a=2, h=16
                )[:, :, :, :, u_lo]
                if P2 % 2 == 0:
                    nc.vector.tensor_copy(dst, po[:, 0:256])
                else:
                    nc.scalar.activation(dst, po[:, 0:256], AF.Copy)

            pending_b = []

            def attn_pair(P2):
                # scores run 2 iterations ahead of the quarter window so the
                # last pairs' softmax chains are done before the flush drains
                # them; P2 >= 64 iterations only drain.
                if P2 < 64:
                    attn_a(P2)
                    pending_b.append(P2)
                    if len(pending_b) > 2:
                        attn_b(pending_b.pop(0))
                elif pending_b:
                    attn_b(pending_b.pop(0))

            def flush_b(upto):
                # attO quarter q must be complete before any final_t(q) is
                # emitted: the PE queue is in-order, so a later-emitted
                # attn_b could never satisfy an earlier final_t (deadlock)
                while pending_b and pending_b[0] < upto:
                    attn_b(pending_b.pop(0))

            def final_w_dma(t2):
                wtile = wp.tile([128, 16, 256], F16, tag="w")
                nc.sync.dma_start(
                    wtile[:],
                    wot[:, t2 * 256 : (t2 + 1) * 256].rearrange(
                        "(k p) c -> p k c", p=128
                    ),
                )
                return wtile

            def final_t(hf, t2, half, wtile):
                t = 2 * t2 + half
                ps = pp.tile([128, 512], F32, tag="pp")
                for sl in range(16):
                    nc.tensor.matmul(
                        ps[:],
                        wtile[:, sl, half * 128 : half * 128 + 128],
                        attO_h[hf][:, sl, :],
                        start=(sl == 0), stop=(sl == 15),
                    )
                ob = op.tile([128, 512], F16, tag="ob")
                nc.vector.tensor_scalar_add(
                    ob[:], ps[:], bias_sb["bo"][:, t : t + 1]
                )
                nc.sync.dma_start(
                    out[t, :, hf * 512 : hf * 512 + 512], ob[:]
                )

            # V projection in two position-phases so attention half 0
            # completes entirely under it with no bunched drain:
            #   phase A (positions 0..511): one psA group per (t2,half) with
            #     both of that iteration's attn_a score pairs interleaved
            #     (pairs 0..31 need only q/k);
            #   phase B (positions 512..1023): psB groups with two attn_b
            #     stages per iteration (their slabs lie in phase A's
            #     positions, complete by then).
            # wv is streamed twice (16MB total) -- the projection-phase DMA
            # bandwidth has 4x headroom, PE cycles are the binding resource.
            for phase in range(2):
                for t2 in range(8):
                    wtile = wp.tile([128, 16, 256], F16, tag="w")
                    nc.sync.dma_start(
                        wtile[:],
                        wvt[:, t2 * 256 : (t2 + 1) * 256].rearrange(
                            "(k p) c -> p k c", p=128
                        ),
                    )
                    for half in range(2):
                        t = 2 * t2 + half
                        ps = pp.tile([128, 512], F32, tag="pp")
                        for k in range(16):
                            w_ap = wtile[:, k, half * 128 : half * 128 + 128]
                            nc.tensor.matmul(
                                ps[:], w_ap,
                                xtb_c[k // 4][:, k % 4, 512 * phase : 512 * phase + 512],
                                start=(k == 0), stop=(k == 15),
                            )
                        copyback(vb, phase, t, ps, "bv")
                        if phase == 0:
                            attn_a(2 * t)
                            pending_b.append(2 * t)
                            attn_a(2 * t + 1)
                            pending_b.append(2 * t + 1)
                        else:
                            attn_b(pending_b.pop(0))
                            attn_b(pending_b.pop(0))
            # attO half 0 is complete (flush is a no-op safety net); first
            # two wot tiles prefetch before pass 0
            wt_next = final_w_dma(0)
            wt_next2 = final_w_dma(1)
            flush_b(32)
            # pairs 32,33 scored early so the shifted interleave window
            # (P2+2) covers every pair exactly once
            for p_early in (32, 33):
                attn_a(p_early)
                pending_b.append(p_early)
            # pass 0 over attO half 0, interleaved 2:1 with the 32 attention
            # pairs of half 1; wot tiles prefetched two chunks ahead
            for t2 in range(8):
                wtile = wt_next
                wt_next = wt_next2
                wt_next2 = final_w_dma((t2 + 2) % 8)
                for half in range(2):
                    final_t(0, t2, half, wtile)
                    attn_pair(32 + 2 * (2 * t2 + half) + 2)
                    attn_pair(32 + 2 * (2 * t2 + half) + 3)
            flush_b(64)
            # pass 1 over attO half 1 has no attention to hide behind: two
            # wot tiles stay in flight so its matmuls never wait on the stream
            for t2 in range(8):
                wtile = wt_next
                wt_next = wt_next2
                if t2 < 6:
                    wt_next2 = final_w_dma(t2 + 2)
                for half in range(2):
                    final_t(1, t2, half, wtile)

    nc.compile()
    return nc


def _get_nc():
    if "nc" not in _CACHE:
        _CACHE["nc"] = _build()
    return _CACHE["nc"]


def make_in_maps(inputs):
    x = np.ascontiguousarray(np.asarray(inputs["x"], dtype=np.float32))
    ws = {k: np.asarray(inputs[k], dtype=np.float32) for k in ("wq", "wk", "wv", "wo")}
    bs = {k: np.asarray(inputs[k], dtype=np.float32) for k in ("bq", "bk", "bv", "bo")}

    xf = x.reshape(B * S, E)
    f16 = lambda a: np.ascontiguousarray(a).astype(np.float16)
    btile = lambda b: np.ascontiguousarray(b.reshape(16, 128).T)
    # score partitions/cols are (h*8 + w) h-major under the block-interleaved
    # qkv layout; same-position mask is w_q == w_k i.e. p%8 == c%8
    ii = np.arange(128) % 8
    mask01 = (ii[:, None] == ii[None, :]).astype(np.float32)
    common = {
        "wqt": f16(ws["wq"].T), "wkt": f16(ws["wk"].T),
        "wvt": f16(ws["wv"].T), "wot": f16(ws["wo"].T),
        "bqt": btile(bs["bq"]), "bkt": btile(bs["bk"]),
        "bvt": btile(bs["bv"]), "bot": btile(bs["bo"]),
        "mask01": mask01, "ident": np.eye(128, dtype=np.float16),
    }
    in_maps = []
    for c in range(NCORES):
        xt_c = f16(xf[c * 1024 : (c + 1) * 1024].T).reshape(16, 128, 1024)
        in_maps.append({"xt": xt_c, **common})
    return in_maps


def assemble(results):
    out = np.empty((B, S, E), np.float32)
    for c in range(NCORES):
        O = results[c]["out"].astype(np.float32)  # [16 t, 128 p, 1024]; col = u*16 + h
        Oc = O.reshape(E, 64, 16)  # [j, u, h]
        tgt = out[c // 2].reshape(16, 128, E)
        v0 = (c % 2) * 64
        tgt[:, v0 : v0 + 64, :] = Oc.transpose(2, 1, 0)
    return out


def kernel(**inputs):
    global LAST_EXEC_NS
    nc = _get_nc()
    res = run_bass_kernel_spmd(nc, make_in_maps(inputs), core_ids=list(range(NCORES)))
    LAST_EXEC_NS = res.exec_time_ns
    return assemble(res.results)
